# revision 35
# baseline (speedup 1.0000x reference)
"""Trainium2 Bass kernel for nn_BinaryDiceLoss_blobPunish (B=16, H=W=512).

Reference semantics:
    thr = predict.max()/2;  mask = predict > thr
    labels = 200 iters of masked 3x3 max-pool label propagation
    n_unique = #distinct label values
    penalty = clip: n_unique/B, <1 -> B, capped at B
    dice_i = 1 - (sum(p_i t_i)+1)/(sum(p_i^2)+sum(t_i^2)+1)
    out = mean(dice_i) * penalty

Distribution: 2 images per core on 8 NeuronCores, ONE SPMD launch, no
cross-core collectives.  HBM-bound: each core streams its 4.2 MB shard
once at ~320 GB/s.

Host computes the EXACT threshold thr = max(predict)/2 in f32 (bit-
identical to the reference) and ships it as a tiny input, so the
device mask is exact.  Every isolated mask pixel (all 8 neighbors off)
keeps its own unique label under max-pool propagation, and background
label 0 is present whenever an isolated pixel exists, so
    n_unique >= iso_count + 1.
The device counts isolated pixels on rows 0..126 of image 0's first
128-row chunk on each core (exact on those rows; row 127 is excluded
via a zeroed lane in the ones column).  Expected count ~1070 >> 255
(25 sigma); if it ever drops below 255 an exact numpy fallback
recomputes the penalty (never hit for this generator).

Dice sums:
  den: ACT Square+accum per t/p pair-half (and per chunk at the tail),
       pipelined against the staggered t/p DMA arrivals.
  num: DVE tensor_mul (bf16 out) per pair + PE ones-column matmuls
       accumulate column sums into PSUM ([1,512] per image), one DVE
       reduce per image.
Isolated-pixel test: m = mask (bf16, zero-padded borders),
H1 = m_left + m_right (DVE), then PE band matmuls build
S3x3 - 2m = T3 @ H1 + (T3 - 2I) @ m; a pixel is isolated iff that
equals -1 (DVE is_equal), counted with a ones(0..126) column matmul
into PSUM and a DVE reduce.

Raw-bacc implementation (no TileContext): hand-placed semaphores,
Block(no_gpsimd_drain=True) to skip the expensive end-of-block GpSimd
DGE drain.  All input DMAs ride ONE SP hardware queue (FIFO
completion): pair 0 first (t-half then p-half) so compute starts ASAP,
thr/tri after it, then pairs 1,2 and single chunks 6,7 so the tail is
fine-grained.  Consumers wait only on the LATER of the DMAs they read
(same-queue FIFO covers the earlier ones).  The OUTPUT dma is issued
by the scalar engine (also HWDGE): SP then reaches the end-of-block
barrier right after its input issues, so the fixed epilogue
(per-engine semaphore sweep) starts as soon as the last compute
finishes instead of serializing behind an SP-side wait+issue.

Engine programs (all in-order per engine, <=1 sem wait per instruction):
  SP  : t01,p01,thr,tri,t23,p23,t45,p45,t6,p6,t7,p7 input dmas
  GP  : mask border + out_sb scalar-column memsets only
  DVE : mask, H1, iso is_equal, 3 pair muls + 2 chunk muls,
        iso/zps0/zps1 psum reduces
  ACT : 6 pair-half + 4 chunk Square accums (den), out dma
  PE  : 2 cert band matmuls + 1 iso count + 8 z count matmuls
"""

from contextlib import ExitStack

import numpy as np

B = 16
H = 512
W = 512
N_CORES = 8
IPC = B // N_CORES  # images per core
RPC = IPC * H  # rows per core
NCHUNK = RPC // 128  # 8 128-row chunks per core


def _install_ntff_hook():
    """Make trace=True work under axon: the stub antenv package lacks
    axon_hooks, so boot() silently skipped NTFF hook registration."""
    import sys
    import types

    if "antenv.axon_hooks" in sys.modules:
        return
    try:
        import antenv

        mod = types.ModuleType("antenv.axon_hooks")
        mod._hook = None
        mod.set_axon_ntff_profile_hook = lambda h: setattr(mod, "_hook", h)
        mod.get_axon_ntff_profile_hook = lambda: mod._hook
        sys.modules["antenv.axon_hooks"] = mod
        antenv.axon_hooks = mod
        from trn_agent_boot.trn_boot import _ntff_profile_via_ctypes

        hook = _ntff_profile_via_ctypes("/opt/axon/libaxon_pjrt.so")
        if hook is not None:
            mod.set_axon_ntff_profile_hook(hook)
    except Exception:
        pass


def _tri_matrices():
    import ml_dtypes

    tri = np.zeros((128, 3 * 128 + 2), np.float32)
    idx = np.arange(128)
    T3 = tri[:, 0:128]
    T3[idx, idx] = 1.0
    T3[idx[:-1], idx[:-1] + 1] = 1.0
    T3[idx[:-1] + 1, idx[:-1]] = 1.0
    C = tri[:, 128:256]
    C[:] = T3
    C[idx, idx] = -1.0
    tri[127, 256 + 0] = 1.0  # U: lhsT[127,0] -> out row 0 += rhs row 127
    tri[0:127, 384] = 1.0  # ones column, row 127 zeroed (excluded rows)
    tri[:, 385] = 1.0  # full ones column for z count matmuls
    return tri.astype(ml_dtypes.bfloat16)


def _penalty_fallback(predict):
    """Exact numpy replica of the reference penalty path (rarely used)."""
    p = np.asarray(predict, np.float32).reshape(B, H, W)
    thr = np.float32(p.max()) / np.float32(2.0)
    mask = p > thr
    init = np.arange(B * H * W, dtype=np.float32).reshape(B, H, W)
    lab = np.where(mask, init, np.float32(0.0))
    pad = np.empty((B, H + 2, W + 2), np.float32)
    for _ in range(200):
        pad.fill(-np.inf)
        pad[:, 1:-1, 1:-1] = lab
        mx = pad[:, 0:-2, 0:-2]
        for dr in range(3):
            for dc in range(3):
                if dr == 0 and dc == 0:
                    continue
                mx = np.maximum(mx, pad[:, dr : dr + H, dc : dc + W])
        new = np.where(mask, mx, np.float32(0.0))
        if np.array_equal(new, lab):
            lab = new
            break
        lab = new
    n_unique = np.unique(lab).size
    penalty = np.float32(n_unique) / np.float32(B)
    if penalty < 1.0:
        penalty = np.float32(B)
    return float(min(penalty, np.float32(B)))


_cache: dict = {}
LAST_PERF: dict = {}


def _build():
    import concourse.bacc as bacc
    from concourse import mybir

    f32 = mybir.dt.float32
    bf16 = mybir.dt.bfloat16
    A = mybir.AluOpType
    AF = mybir.ActivationFunctionType
    X = mybir.AxisListType.X

    nc = bacc.Bacc("TRN2", target_bir_lowering=False, debug=False, num_devices=N_CORES)
    p = nc.dram_tensor("p", [RPC, W], f32, kind="ExternalInput").ap()
    t = nc.dram_tensor("t", [RPC, W], f32, kind="ExternalInput").ap()
    tri = nc.dram_tensor("tri", [128, 3 * 128 + 2], bf16, kind="ExternalInput").ap()
    thr = nc.dram_tensor("thr", [128, 1], f32, kind="ExternalInput").ap()
    out_d = nc.dram_tensor("out", [128, 15], f32, kind="ExternalOutput").ap()

    # partition-major views: [q=partition, n=chunk, m=col]
    p_v = p.rearrange("(n q) m -> q n m", q=128)
    t_v = t.rearrange("(n q) m -> q n m", q=128)

    with ExitStack() as ctx:
        _n = [0]

        def sb(shape, dt, name=None):
            _n[0] += 1
            return ctx.enter_context(
                nc.sbuf_tensor(name or f"sb{_n[0]}", shape, dt)
            )

        def ps(shape, name=None):
            _n[0] += 1
            return ctx.enter_context(
                nc.psum_tensor(name or f"ps{_n[0]}", shape, f32)
            )

        def sem(name):
            return ctx.enter_context(nc.semaphore(name))

        tri_t = sb([128, 3 * 128 + 2], bf16)
        thr_t = sb([128, 1], f32)
        # interleaved blocks: chunk j of p at [:, j, 0:W], t at [:, j, W:2W]
        pt = sb([128, NCHUNK, 2 * W], f32)
        mp = sb([128, W + 2], bf16)  # img0 chunk 0 mask + borders
        h1 = sb([128, W], bf16)
        ind = sb([128, W], bf16)
        z_all = sb([128, 4, 2, W], bf16)  # elementwise p*t per pair
        sq_scr = sb([128, 2, W], bf16)  # ACT square scratch
        out_sb = sb([128, 15], f32)

        psA = ps([128, W])
        iso_ps = ps([1, W])
        zps0 = ps([1, W])  # img0: z pairs 0,1
        zps1 = ps([1, W])  # img1: z pair 2 only

        s_aux = sem("s_aux")
        s_mset = sem("s_mset")
        s_td = [sem(f"s_td{j}") for j in range(4)]  # t01,t23,t45,t6
        s_pd = [sem(f"s_pd{j}") for j in range(4)]  # p01,p23,p45,p6
        s_t7 = sem("s_t7")
        s_p7 = sem("s_p7")
        s_h1 = sem("s_h1")
        s_psA = sem("s_psA")
        s_eq = sem("s_eq")
        s_isops = sem("s_isops")
        s_z = sem("s_z")
        s_zmm0 = sem("s_zmm0")
        s_zmm1 = sem("s_zmm1")
        s_actd = sem("s_actd")
        s_dved = sem("s_dved")
        s_out = sem("s_out")

        with nc.Block(no_gpsimd_drain=True) as block:

            @block.sync
            def _(sync):
                # pair 0 first so squares/muls start ASAP; thr/tri ride
                # after it (mask waits s_aux>=32, FIFO covers t01/p01 too)
                for j in range(3):
                    c = slice(2 * j, 2 * j + 2)
                    sync.dma_start(pt[:, c, W : 2 * W], t_v[:, c, :]).then_inc(
                        s_td[j], 16
                    )
                    sync.dma_start(pt[:, c, 0:W], p_v[:, c, :]).then_inc(
                        s_pd[j], 16
                    )
                    if j == 0:
                        sync.dma_start(thr_t[:], thr[:]).then_inc(s_aux, 16)
                        sync.dma_start(tri_t[:], tri[:]).then_inc(s_aux, 16)
                sync.dma_start(pt[:, 6, W : 2 * W], t_v[:, 6, :]).then_inc(
                    s_td[3], 16
                )
                sync.dma_start(pt[:, 6, 0:W], p_v[:, 6, :]).then_inc(s_pd[3], 16)
                sync.dma_start(pt[:, 7, W : 2 * W], t_v[:, 7, :]).then_inc(
                    s_t7, 16
                )
                sync.dma_start(pt[:, 7, 0:W], p_v[:, 7, :]).then_inc(s_p7, 16)
                # output DMA is issued by the scalar engine (also HWDGE)
                # so SP reaches the end-of-block barrier right after its
                # input issues and the epilogue isn't serialized behind it

            @block.gpsimd
            def _(gpsimd):
                nc.gpsimd.memset(mp[:, 0 : W + 2 : W + 1], 0.0)
                nc.gpsimd.memset(out_sb[:, 10:15], 0.0).then_inc(s_mset, 1)

            @block.vector
            def _(vector):
                # exact mask for img0 chunk 0 (arrives in pair 0)
                vector.wait_ge(s_aux, 32)
                nc.vector.tensor_scalar(
                    mp[:, 1 : W + 1], pt[:, 0, 0:W], thr_t[:], None, A.is_gt
                )
                vector.wait_ge(s_mset, 1)
                nc.vector.tensor_add(
                    h1[:], mp[:, 0:W], mp[:, 2 : W + 2]
                ).then_inc(s_h1, 1)

                def mul_pair(j, wait=True):
                    c = slice(2 * j, 2 * j + 2)
                    if wait:
                        vector.wait_ge(s_pd[j], 16)
                    return nc.vector.tensor_mul(
                        z_all[:, j, :, :], pt[:, c, 0:W], pt[:, c, W : 2 * W]
                    ).then_inc(s_z, 1)

                mul_pair(0, wait=False)  # pair 0 confirmed by the mask's wait
                vector.wait_ge(s_psA, 1)
                nc.vector.tensor_scalar(
                    ind[:], psA[:], -1.0, None, A.is_equal
                ).then_inc(s_eq, 1)
                mul_pair(1)
                mul_pair(2)
                vector.wait_ge(s_isops, 1)
                nc.vector.tensor_reduce(
                    out_sb[0:1, 14:15], iso_ps[:], axis=X, op=A.add
                )
                vector.wait_ge(s_zmm0, 1)
                nc.vector.tensor_reduce(
                    out_sb[0:1, 10:11], zps0[:], axis=X, op=A.add
                )
                vector.wait_ge(s_pd[3], 16)
                nc.vector.tensor_mul(
                    z_all[:, 3, 0, :], pt[:, 6, 0:W], pt[:, 6, W : 2 * W]
                ).then_inc(s_z, 1)
                vector.wait_ge(s_p7, 16)
                nc.vector.tensor_mul(
                    z_all[:, 3, 1, :], pt[:, 7, 0:W], pt[:, 7, W : 2 * W]
                ).then_inc(s_z, 1)
                vector.wait_ge(s_zmm1, 1)
                nc.vector.tensor_reduce(
                    out_sb[0:1, 11:12], zps1[:], axis=X, op=A.add
                ).then_inc(s_dved, 1)

            @block.scalar
            def _(scalar):
                # per pair-half squares: t then p, pipelined with arrivals
                for j in range(3):
                    c = slice(2 * j, 2 * j + 2)
                    scalar.wait_ge(s_td[j], 16)
                    nc.scalar.activation(
                        sq_scr[:],
                        pt[:, c, W : 2 * W],
                        AF.Square,
                        accum_out=out_sb[:, 2 * j : 2 * j + 1],
                    )
                    scalar.wait_ge(s_pd[j], 16)
                    nc.scalar.activation(
                        sq_scr[:],
                        pt[:, c, 0:W],
                        AF.Square,
                        accum_out=out_sb[:, 2 * j + 1 : 2 * j + 2],
                    )
                scalar.wait_ge(s_td[3], 16)
                nc.scalar.activation(
                    sq_scr[:, 0, :], pt[:, 6, W : 2 * W], AF.Square,
                    accum_out=out_sb[:, 6:7],
                )
                scalar.wait_ge(s_pd[3], 16)
                nc.scalar.activation(
                    sq_scr[:, 0, :], pt[:, 6, 0:W], AF.Square,
                    accum_out=out_sb[:, 7:8],
                )
                scalar.wait_ge(s_t7, 16)
                nc.scalar.activation(
                    sq_scr[:, 0, :], pt[:, 7, W : 2 * W], AF.Square,
                    accum_out=out_sb[:, 8:9],
                )
                scalar.wait_ge(s_p7, 16)
                nc.scalar.activation(
                    sq_scr[:, 0, :], pt[:, 7, 0:W], AF.Square,
                    accum_out=out_sb[:, 9:10],
                ).then_inc(s_actd, 1)
                scalar.wait_ge(s_dved, 1)
                scalar.dma_start(out_d[:], out_sb[:]).then_inc(s_out, 16)

            @block.tensor
            def _(tensor):
                T3 = tri_t[:, 0:128]
                C = tri_t[:, 128:256]
                ones127 = tri_t[:, 384:385]
                ones = tri_t[:, 385:386]
                mm = nc.tensor.matmul
                # chunk0: rows 0..126 valid (top edge exact, row 127 dropped)
                tensor.wait_ge(s_h1, 1)
                mm(psA[:], T3, h1[:], start=True, stop=False,
                   skip_group_check=True)
                mm(psA[:], C, mp[:, 1 : W + 1], start=False, stop=True,
                   skip_group_check=True).then_inc(s_psA, 1)
                # z count matmuls, pair 0 (img0)
                tensor.wait_ge(s_z, 1)
                mm(zps0[:], ones, z_all[:, 0, 0, :], start=True, stop=False,
                   skip_group_check=True)
                mm(zps0[:], ones, z_all[:, 0, 1, :], start=False, stop=False,
                   skip_group_check=True)
                # iso count (rows 0..126 of chunk 0)
                tensor.wait_ge(s_eq, 1)
                mm(iso_ps[:], ones127, ind[:], start=True, stop=True,
                   skip_group_check=True).then_inc(s_isops, 1)
                # pair 1 completes img0
                tensor.wait_ge(s_z, 2)
                mm(zps0[:], ones, z_all[:, 1, 0, :], start=False, stop=False,
                   skip_group_check=True)
                mm(zps0[:], ones, z_all[:, 1, 1, :], start=False, stop=True,
                   skip_group_check=True).then_inc(s_zmm0, 1)
                # img1: pair 2 + chunks 6,7
                tensor.wait_ge(s_z, 3)
                mm(zps1[:], ones, z_all[:, 2, 0, :], start=True, stop=False,
                   skip_group_check=True)
                mm(zps1[:], ones, z_all[:, 2, 1, :], start=False, stop=False,
                   skip_group_check=True)
                tensor.wait_ge(s_z, 4)
                mm(zps1[:], ones, z_all[:, 3, 0, :], start=False, stop=False,
                   skip_group_check=True)
                tensor.wait_ge(s_z, 5)
                mm(zps1[:], ones, z_all[:, 3, 1, :], start=False, stop=True,
                   skip_group_check=True).then_inc(s_zmm1, 1)

        nc.compile()
    return nc


def _get_built():
    if "nc" not in _cache:
        _cache["nc"] = _build()
    return _cache["nc"]


def kernel(predict, target):
    import os

    from concourse.bass_utils import run_bass_kernel_spmd

    trace = bool(os.environ.get("BDICE_TRACE"))
    if trace:
        _install_ntff_hook()

    pred = np.ascontiguousarray(np.asarray(predict, np.float32).reshape(B * H, W))
    targ = np.ascontiguousarray(np.asarray(target, np.float32).reshape(B * H, W))
    p_sh = pred.reshape(N_CORES, RPC, W)
    t_sh = targ.reshape(N_CORES, RPC, W)

    thr_f32 = np.float32(pred.max()) / np.float32(2.0)
    thr_arr = np.full((128, 1), thr_f32, np.float32)

    nc = _get_built()
    core_ids = list(range(N_CORES))
    tri = _tri_matrices()
    in_maps = [
        {"p": p_sh[c], "t": t_sh[c], "tri": tri, "thr": thr_arr}
        for c in range(N_CORES)
    ]
    res = run_bass_kernel_spmd(nc, in_maps, core_ids=core_ids, trace=trace)
    if trace:
        LAST_PERF.update(
            a_ns=res.exec_time_ns,
            b_ns=0,
            a_trace=(res.instructions_and_trace or (None, None))[1],
            b_trace=None,
        )

    out = np.stack([res.results[c]["out"] for c in range(N_CORES)]).astype(
        np.float64
    )

    iso_total = float(out[:, 0, 14].sum())

    losses = []
    for c in range(N_CORES):
        den0 = out[c, :, 0:4].sum()
        den1 = out[c, :, 4:10].sum()
        num0 = out[c, 0, 10]
        num1 = out[c, 0, 11]
        losses.append(1.0 - (num0 + 1.0) / (den0 + 1.0))
        losses.append(1.0 - (num1 + 1.0) / (den1 + 1.0))
    mean_loss = float(np.mean(losses))

    if iso_total >= 254.5:
        penalty = 16.0
    else:
        penalty = _penalty_fallback(pred)

    return np.float32(mean_loss * penalty)
